# revision 5
# baseline (speedup 1.0000x reference)
"""Trainium2 Bass kernel for HalfHadamardTrustQuantizer.

Computation (forward value of the reference, which collapses to xq):
  x_had = blockwise-64 Hadamard rotation of channels:  (B,C,H,W), C=512 = 8 groups of 64
  std   = sqrt(mean(x_had^2)) per sample  (== RMS of x by orthogonality)
  scale = OPT*std + 1e-8 ; step = 2*scale/15
  xq    = round(clip(x_had,-scale,scale)/step + 0.5)*step - step/2

Sharding: data-parallel over batch; 2 samples per core on 8 cores.

Per-core pipeline (2 samples of (512, 3136) fp32):
  loads (f32): split 1024 cols on the SP queue + 2112 cols on the ACT queue
  phase A per row: ACT Square+accum_out -> per-partition sumsq (s1 rows 2-3
         use DVE to unload ACT in the middle phase); DVE copy to fp32r
         (fp32 with 11-bit mantissa; the PE matmul runs 1 cycle/row on an
         fp32r moving operand vs 4 cycles/row for fp32)
  scalars: PE ones-matmul cross-partition reduce+broadcast; ACT Sqrt + one DVE
           Newton step; reciprocal for 1/step
  phase B: PE fp32r matmul with blockdiag(aux,aux) weights (N=512 chunks);
           ACT Identity(scale=1/step, bias=0.5) PSUM -> int32 (RNE round fused);
           Pool int clip (min 8, max -7); DVE affine int->f32 (*step - step/2);
           stores: [0,2048) on the Pool SWDGE queue, [2048,3136) on SP
"""

import numpy as np
from contextlib import ExitStack

B, C, HH, WW = 16, 512, 56, 56
R = HH * WW            # 3136 spatial
NCORES = 8
S = B // NCORES        # samples per core
NB = C // 128          # block-rows per sample
N_ELEM = C * R         # per-sample reduction size
OPT = 2.513930578568423
INV_N = np.float64(1.0) / np.float64(N_ELEM)   # cast at use
TWO_15 = np.float32(2.0) / np.float32(15.0)

LD_SPLIT = 1024        # cols loaded via SP queue; rest via ACT queue

_CACHE = {}


def _build_program():
    import concourse.bacc as bacc
    import concourse.tile as tile
    import concourse.mybir as mybir

    AF = mybir.ActivationFunctionType
    OP = mybir.AluOpType
    f32 = mybir.dt.float32
    f32r = mybir.dt.float32r
    i32 = mybir.dt.int32

    nc = bacc.Bacc("TRN2", target_bir_lowering=False, debug=False,
                   num_devices=NCORES)
    x = nc.dram_tensor("x", [S * C, R], f32, kind="ExternalInput").ap()
    w = nc.dram_tensor("w", [128, 128], f32, kind="ExternalInput").ap()
    y = nc.dram_tensor("y", [S * C, R], f32, kind="ExternalOutput").ap()

    # matmul chunks grouped in pairs into 2-bank PSUM tiles; ACT drains a
    # whole tile at once
    TILES = [(0, [512, 512]), (1024, [512, 512]), (2048, [512, 512]),
             (3072, [64])]
    # output halves for clip/affine/store, aligned to drain tiles
    HALVES = [(0, 2048), (2048, 1088)]

    with tile.TileContext(nc) as tc, ExitStack() as ctx:
        xp = ctx.enter_context(tc.tile_pool(name="xp", bufs=3))
        xrp = ctx.enter_context(tc.tile_pool(name="xrp", bufs=8))
        cn = ctx.enter_context(tc.tile_pool(name="cn", bufs=1))
        sq = ctx.enter_context(tc.tile_pool(name="sq", bufs=1))
        ac = ctx.enter_context(tc.tile_pool(name="ac", bufs=2))
        sc = ctx.enter_context(tc.tile_pool(name="sc", bufs=2))
        ip = ctx.enter_context(tc.tile_pool(name="ip", bufs=2))
        op_ = ctx.enter_context(tc.tile_pool(name="op", bufs=2))
        pp = ctx.enter_context(tc.tile_pool(name="pp", bufs=4, space="PSUM"))

        wt = cn.tile([128, 128], f32r, tag="w")
        nc.gpsimd.dma_start(wt[:], w[:])
        ones = cn.tile([128, 128], f32, tag="ones")
        nc.gpsimd.memset(ones[:], 1.0)
        half = cn.tile([128, 1], f32, tag="half")
        nc.gpsimd.memset(half[:], 0.5)

        sq_scr = sq.tile([128, R], f32, tag="sqscr")

        xrs = {}
        scal = {}

        def phase_a_row(s, b, sq_eng):
            # load one block-row split across the SP and ACT HWDGE queues,
            # square-accumulate it (ACT or DVE), and produce the fp32r copy
            # the PE will consume (DVE pass; rounds off 12 mantissa bits)
            xt = xp.tile([128, R], f32, tag="xrow")
            r0 = x[s * C + b * 128:s * C + (b + 1) * 128, :]
            nc.sync.dma_start(xt[:, 0:LD_SPLIT], r0[:, 0:LD_SPLIT])
            nc.scalar.dma_start(xt[:, LD_SPLIT:R], r0[:, LD_SPLIT:R])
            if sq_eng == "act":
                nc.scalar.activation(sq_scr[:], xt[:], AF.Square,
                                     accum_out=parts[s][:, b:b + 1])
            else:
                nc.vector.scalar_tensor_tensor(sq_scr[:], xt[:], 1.0, xt[:],
                                               OP.mult, OP.mult,
                                               accum_out=parts[s][:, b:b + 1])
            xr = xrp.tile([128, R], f32r, tag="xr")
            nc.vector.tensor_scalar_mul(xr[:], xt[:], 1.0)
            xrs[(s, b)] = xr

        def sample_scalars(s):
            # ---- per-sample scalars ----
            part = parts[s]
            red = sc.tile([128, 1], f32, tag="red")
            nc.vector.reduce_sum(red[:], part[:], axis=mybir.AxisListType.X)
            tot = pp.tile([128, 1024], f32, tag="pchunk")
            tot = tot[:, 0:1]
            nc.tensor.matmul(tot[:], ones[:], red[:], start=True, stop=True)
            std0 = sc.tile([128, 1], f32, tag="std0")
            nc.scalar.activation(std0[:], tot[:], AF.Sqrt, scale=float(INV_N))
            # one Newton step: std1 = 0.5*std0 + (0.5/N)*(tot * (1/(std0+eps)))
            std0p = sc.tile([128, 1], f32, tag="std0p")
            nc.vector.tensor_scalar_add(std0p[:], std0[:], 1e-30)
            rstd = sc.tile([128, 1], f32, tag="rstd")
            nc.vector.reciprocal(rstd[:], std0p[:])
            t1 = sc.tile([128, 1], f32, tag="t1")
            nc.vector.tensor_tensor(t1[:], tot[:], rstd[:], OP.mult)
            hs = sc.tile([128, 1], f32, tag="hs")
            nc.vector.tensor_scalar_mul(hs[:], std0[:], 0.5)
            std1 = sc.tile([128, 1], f32, tag="std1")
            nc.vector.scalar_tensor_tensor(std1[:], t1[:], float(0.5 * INV_N),
                                           hs[:], OP.mult, OP.add)
            scale_t = sc.tile([128, 1], f32, tag="scale")
            nc.vector.tensor_scalar(scale_t[:], std1[:], float(OPT), 1e-8,
                                    OP.mult, OP.add)
            step = sc.tile([128, 1], f32, tag="step")
            nc.vector.tensor_scalar_mul(step[:], scale_t[:], float(TWO_15))
            inv = sc.tile([128, 1], f32, tag="inv")
            nc.vector.reciprocal(inv[:], step[:])
            hstep = sc.tile([128, 1], f32, tag="hstep")
            nc.vector.tensor_scalar_mul(hstep[:], step[:], 0.5)
            scal[s] = (inv, step, hstep)

        def phase_b_row(s, b):
            inv, step, hstep = scal[s]
            xr = xrs.pop((s, b))
            # ---- phase B: rotate + quantize + store ----
            irow = ip.tile([128, R], i32, tag="irow")
            for off, chunks in TILES:
                tw = sum(chunks)
                pm = pp.tile([128, 1024], f32, tag="pchunk")
                co = 0
                for ch in chunks:
                    nc.tensor.matmul(pm[:, co:co + ch], wt[:],
                                     xr[:, off + co:off + co + ch],
                                     start=True, stop=True)
                    co += ch
                nc.scalar.activation(irow[:, off:off + tw],
                                     pm[:, :tw], AF.Identity,
                                     bias=half[:], scale=inv[:])
            orow = op_.tile([128, R], f32, tag="orow")
            for off, w_ in HALVES:
                nc.gpsimd.tensor_scalar(irow[:, off:off + w_],
                                        irow[:, off:off + w_], 8, -7,
                                        OP.min, OP.max)
                nc.vector.tensor_scalar(orow[:, off:off + w_],
                                        irow[:, off:off + w_],
                                        step[:], hstep[:],
                                        OP.mult, OP.subtract)
                eng = nc.gpsimd if off == 0 else nc.sync
                eng.dma_start(
                    y[s * C + b * 128:s * C + (b + 1) * 128,
                      off:off + w_], orow[:, off:off + w_])

        # ---- pipelined emission order ----
        # s0 loads (+ square/convert), s0 scalars, then s1 loads interleaved
        # with s0 phase-B rows, then s1 scalars and s1 phase B.  s1 squares
        # rows 0-1 on ACT, rows 2-3 on DVE: balances the middle phase where
        # ACT also drains s0's PSUM.
        parts = {}
        for s in range(S):
            part_t = ac.tile([128, NB], f32, tag=f"part{s}", name=f"part{s}")
            parts[s] = part_t
        for b in range(NB):
            phase_a_row(0, b, "act")
        sample_scalars(0)
        for b in range(NB):
            phase_a_row(1, b, "act" if b < 2 else "dve")
            phase_b_row(0, b)
        sample_scalars(1)
        for b in range(NB):
            phase_b_row(1, b)
    nc.compile()
    return nc


def _get_program():
    if "nc" not in _CACHE:
        _CACHE["nc"] = _build_program()
    return _CACHE["nc"]


def kernel(x: np.ndarray, aux_matrix: np.ndarray) -> np.ndarray:
    from concourse.bass_utils import run_bass_kernel_spmd

    x = np.ascontiguousarray(x, dtype=np.float32)
    aux = np.ascontiguousarray(aux_matrix, dtype=np.float32)
    w128 = np.zeros((128, 128), dtype=np.float32)
    w128[:64, :64] = aux
    w128[64:, 64:] = aux

    nc = _get_program()
    in_maps = [
        {"x": x[c * S:(c + 1) * S].reshape(S * C, R), "w": w128}
        for c in range(NCORES)
    ]
    res = run_bass_kernel_spmd(nc, in_maps, list(range(NCORES)))
    out = np.empty((B, C, HH, WW), dtype=np.float32)
    for c in range(NCORES):
        out[c * S:(c + 1) * S] = res.results[c]["y"].reshape(S, C, HH, WW)
    return out


# revision 7
# speedup vs baseline: 1.0519x; 1.0519x over previous
"""Trainium2 Bass kernel for HalfHadamardTrustQuantizer.

Computation (forward value of the reference, which collapses to xq):
  x_had = blockwise-64 Hadamard rotation of channels:  (B,C,H,W), C=512 = 8 groups of 64
  std   = sqrt(mean(x_had^2)) per sample  (== RMS of x by orthogonality)
  scale = OPT*std + 1e-8 ; step = 2*scale/15
  xq    = round(clip(x_had,-scale,scale)/step + 0.5)*step - step/2

Sharding: data-parallel over batch; 2 samples per core on 8 cores.

Per-core pipeline (2 samples of (512, 3136) fp32):
  loads (f32): split 1024 cols on the SP queue + 2112 cols on the ACT queue
  phase A per row: ACT Square+accum_out -> per-partition sumsq (s1 rows 2-3
         use DVE to unload ACT in the middle phase); DVE copy to fp32r
         (fp32 with 11-bit mantissa; the PE matmul runs 1 cycle/row on an
         fp32r moving operand vs 4 cycles/row for fp32)
  scalars: PE ones-matmul cross-partition reduce+broadcast; ACT Sqrt + one DVE
           Newton step; reciprocal for 1/step
  phase B: PE fp32r matmul with blockdiag(aux,aux) weights (N=512 chunks);
           ACT Identity(scale=1/step, bias=0.5) PSUM -> int32 (RNE round fused);
           Pool int clip (min 8, max -7); DVE affine int->f32 (*step - step/2);
           stores: [0,2048) on the Pool SWDGE queue, [2048,3136) on SP
"""

import numpy as np
from contextlib import ExitStack

B, C, HH, WW = 16, 512, 56, 56
R = HH * WW            # 3136 spatial
NCORES = 8
S = B // NCORES        # samples per core
NB = C // 128          # block-rows per sample
N_ELEM = C * R         # per-sample reduction size
OPT = 2.513930578568423
INV_N = np.float64(1.0) / np.float64(N_ELEM)   # cast at use
TWO_15 = np.float32(2.0) / np.float32(15.0)

LD_SPLIT = 1024        # cols loaded via SP queue; rest via ACT queue

_CACHE = {}


def _build_program():
    import concourse.bacc as bacc
    import concourse.tile as tile
    import concourse.mybir as mybir

    AF = mybir.ActivationFunctionType
    OP = mybir.AluOpType
    f32 = mybir.dt.float32
    f32r = mybir.dt.float32r
    i32 = mybir.dt.int32

    nc = bacc.Bacc("TRN2", target_bir_lowering=False, debug=False,
                   num_devices=NCORES)
    x = nc.dram_tensor("x", [S * C, R], f32, kind="ExternalInput").ap()
    w = nc.dram_tensor("w", [128, 128], f32, kind="ExternalInput").ap()
    y = nc.dram_tensor("y", [S * C, R], f32, kind="ExternalOutput").ap()

    # matmul chunks grouped in pairs into 2-bank PSUM tiles; ACT drains a
    # whole tile at once
    TILES = [(0, [512, 512]), (1024, [512, 512]), (2048, [512, 512]),
             (3072, [64])]
    # output halves for clip/affine/store, aligned to drain tiles
    HALVES = [(0, 2048), (2048, 1088)]

    with tile.TileContext(nc) as tc, ExitStack() as ctx:
        xp = ctx.enter_context(tc.tile_pool(name="xp", bufs=3))
        xrp = ctx.enter_context(tc.tile_pool(name="xrp", bufs=8))
        cn = ctx.enter_context(tc.tile_pool(name="cn", bufs=1))
        sq = ctx.enter_context(tc.tile_pool(name="sq", bufs=1))
        ac = ctx.enter_context(tc.tile_pool(name="ac", bufs=2))
        sc = ctx.enter_context(tc.tile_pool(name="sc", bufs=2))
        ip = ctx.enter_context(tc.tile_pool(name="ip", bufs=2))
        op_ = ctx.enter_context(tc.tile_pool(name="op", bufs=2))
        pp = ctx.enter_context(tc.tile_pool(name="pp", bufs=4, space="PSUM"))

        wt = cn.tile([128, 128], f32r, tag="w")
        nc.gpsimd.dma_start(wt[:], w[:])
        ones = cn.tile([128, 128], f32, tag="ones")
        nc.gpsimd.memset(ones[:], 1.0)
        half = cn.tile([128, 1], f32, tag="half")
        nc.gpsimd.memset(half[:], 0.5)

        sq_scr = sq.tile([128, R], f32, tag="sqscr")

        xrs = {}
        scal = {}

        def phase_a_row(s, b, sq_eng, ld_eng):
            # load one block-row as a single full-row DMA (12544B per
            # partition -- descriptors this large keep all 16 DMA engines
            # fed; split rows cap a queue at ~8 in-flight), then
            # square-accumulate it (ACT or DVE) and produce the fp32r copy
            # the PE will consume (DVE pass; rounds off 12 mantissa bits)
            xt = xp.tile([128, R], f32, tag="xrow")
            r0 = x[s * C + b * 128:s * C + (b + 1) * 128, :]
            ld_eng.dma_start(xt[:], r0[:])
            if sq_eng == "act":
                nc.scalar.activation(sq_scr[:], xt[:], AF.Square,
                                     accum_out=parts[s][:, b:b + 1])
            else:
                nc.vector.scalar_tensor_tensor(sq_scr[:], xt[:], 1.0, xt[:],
                                               OP.mult, OP.mult,
                                               accum_out=parts[s][:, b:b + 1])
            xr = xrp.tile([128, R], f32r, tag="xr")
            nc.vector.tensor_scalar_mul(xr[:], xt[:], 1.0)
            xrs[(s, b)] = xr

        def sample_scalars(s):
            # ---- per-sample scalars ----
            part = parts[s]
            red = sc.tile([128, 1], f32, tag="red")
            nc.vector.reduce_sum(red[:], part[:], axis=mybir.AxisListType.X)
            tot = pp.tile([128, 1024], f32, tag="pchunk")
            tot = tot[:, 0:1]
            nc.tensor.matmul(tot[:], ones[:], red[:], start=True, stop=True)
            std0 = sc.tile([128, 1], f32, tag="std0")
            nc.scalar.activation(std0[:], tot[:], AF.Sqrt, scale=float(INV_N))
            # one Newton step: std1 = 0.5*std0 + (0.5/N)*(tot * (1/(std0+eps)))
            std0p = sc.tile([128, 1], f32, tag="std0p")
            nc.vector.tensor_scalar_add(std0p[:], std0[:], 1e-30)
            rstd = sc.tile([128, 1], f32, tag="rstd")
            nc.vector.reciprocal(rstd[:], std0p[:])
            t1 = sc.tile([128, 1], f32, tag="t1")
            nc.vector.tensor_tensor(t1[:], tot[:], rstd[:], OP.mult)
            hs = sc.tile([128, 1], f32, tag="hs")
            nc.vector.tensor_scalar_mul(hs[:], std0[:], 0.5)
            std1 = sc.tile([128, 1], f32, tag="std1")
            nc.vector.scalar_tensor_tensor(std1[:], t1[:], float(0.5 * INV_N),
                                           hs[:], OP.mult, OP.add)
            scale_t = sc.tile([128, 1], f32, tag="scale")
            nc.vector.tensor_scalar(scale_t[:], std1[:], float(OPT), 1e-8,
                                    OP.mult, OP.add)
            step = sc.tile([128, 1], f32, tag="step")
            nc.vector.tensor_scalar_mul(step[:], scale_t[:], float(TWO_15))
            inv = sc.tile([128, 1], f32, tag="inv")
            nc.vector.reciprocal(inv[:], step[:])
            hstep = sc.tile([128, 1], f32, tag="hstep")
            nc.vector.tensor_scalar_mul(hstep[:], step[:], 0.5)
            scal[s] = (inv, step, hstep)

        def phase_b_row(s, b, st_eng):
            inv, step, hstep = scal[s]
            xr = xrs.pop((s, b))
            # ---- phase B: rotate + quantize + store (one full-row DMA) ----
            irow = ip.tile([128, R], i32, tag="irow")
            for off, chunks in TILES:
                tw = sum(chunks)
                pm = pp.tile([128, 1024], f32, tag="pchunk")
                co = 0
                for ch in chunks:
                    nc.tensor.matmul(pm[:, co:co + ch], wt[:],
                                     xr[:, off + co:off + co + ch],
                                     start=True, stop=True)
                    co += ch
                nc.scalar.activation(irow[:, off:off + tw],
                                     pm[:, :tw], AF.Identity,
                                     bias=half[:], scale=inv[:])
            orow = op_.tile([128, R], f32, tag="orow")
            for off, w_ in HALVES:
                nc.gpsimd.tensor_scalar(irow[:, off:off + w_],
                                        irow[:, off:off + w_], 8, -7,
                                        OP.min, OP.max)
                nc.vector.tensor_scalar(orow[:, off:off + w_],
                                        irow[:, off:off + w_],
                                        step[:], hstep[:],
                                        OP.mult, OP.subtract)
            st_eng.dma_start(
                y[s * C + b * 128:s * C + (b + 1) * 128, :], orow[:])

        # ---- pipelined emission order ----
        # s0 loads (+ square/convert), s0 scalars, then s1 loads interleaved
        # with s0 phase-B rows, then s1 scalars and s1 phase B.  s1 squares
        # rows 0-1 on ACT, rows 2-3 on DVE: balances the middle phase where
        # ACT also drains s0's PSUM.  Loads alternate SP/ACT queues; stores
        # go mostly to the Pool SWDGE queue (3 rows/sample) + SP (1 row).
        parts = {}
        for s in range(S):
            part_t = ac.tile([128, NB], f32, tag=f"part{s}", name=f"part{s}")
            parts[s] = part_t
        LD0 = [nc.sync, nc.scalar, nc.sync, nc.scalar]
        LD1 = [nc.sync, nc.scalar, nc.scalar, nc.scalar]
        ST = [nc.gpsimd, nc.gpsimd, nc.gpsimd, nc.sync]
        for b in range(NB):
            phase_a_row(0, b, "act", LD0[b])
        sample_scalars(0)
        for b in range(NB):
            phase_a_row(1, b, "act" if b < 2 else "dve", LD1[b])
            phase_b_row(0, b, ST[b])
        sample_scalars(1)
        for b in range(NB):
            phase_b_row(1, b, ST[b])
    nc.compile()
    return nc


def _get_program():
    if "nc" not in _CACHE:
        _CACHE["nc"] = _build_program()
    return _CACHE["nc"]


def kernel(x: np.ndarray, aux_matrix: np.ndarray) -> np.ndarray:
    from concourse.bass_utils import run_bass_kernel_spmd

    x = np.ascontiguousarray(x, dtype=np.float32)
    aux = np.ascontiguousarray(aux_matrix, dtype=np.float32)
    w128 = np.zeros((128, 128), dtype=np.float32)
    w128[:64, :64] = aux
    w128[64:, 64:] = aux

    nc = _get_program()
    in_maps = [
        {"x": x[c * S:(c + 1) * S].reshape(S * C, R), "w": w128}
        for c in range(NCORES)
    ]
    res = run_bass_kernel_spmd(nc, in_maps, list(range(NCORES)))
    out = np.empty((B, C, HH, WW), dtype=np.float32)
    for c in range(NCORES):
        out[c * S:(c + 1) * S] = res.results[c]["y"].reshape(S, C, HH, WW)
    return out
